# revision 32
# baseline (speedup 1.0000x reference)
"""MultiHeadAttention Trainium2 kernel.

Full inputs -> shard over 8 NeuronCores as (batch, head-group):
core c handles batch c//2 and head-group c%2 (8 of 16 heads, Megatron-style
tensor parallel over heads for the QKV/proj weights). Each core returns a
partial projection output [S, D]; host sums the 2 partials per batch and adds
the biases that commute to the end (v-bias and proj bias).

Device-side layout choices (per core):
  - x is loaded transposed (xT [D, S], fp16) via DMA-transpose.
  - K and Q are produced TRANSPOSED ([e, s] per head, heads paired on
    partitions 0-63 / 64-127) so the score matmul S^T = K^T.T-style runs
    row-tiled (2 heads concurrently in the 128x128 PE array, contract=64).
  - V is produced in natural [s, e] layout so PV runs col-tiled (2 heads
    concurrently, contract=128).
  - exp() runs on ScalarE directly from PSUM with the 1/sqrt(e) scale and the
    attention-mask bias folded into the activation's free affine.
  - softmax denominator: fp16 pairwise-tree adds on VectorE (exact w.r.t. the
    rounded weights the PV matmul consumes), then a ones-matmul reduces the
    128 partitions and replicates the result across partitions for the
    normalize multiply.
"""

import os

import numpy as np

import concourse.bass as bass
import concourse.mybir as mybir
import concourse.tile as tile
from concourse import bacc
from concourse.bass_utils import run_bass_kernel_spmd

B, S, D, H, E = 4, 2048, 1024, 16, 64
G = 2                # head groups (cores per batch)
HL = H // G          # local heads per core = 8
NPAIR = HL // 2      # 4 head pairs
DL = HL * E          # 512 local head dims
P = 128
QT = 1024            # q-tile width in the attention loop
NKB = S // P         # 16 key blocks
DC = D // P          # 8 contraction chunks of the model dim
DCL = DL // P        # 4 local-dim chunks for the projection
F16 = mybir.dt.float16
F32 = mybir.dt.float32

LAST_RESULTS = None
_CACHE = {}


def _install_ntff_hook():
    """Synthesize antenv.axon_hooks (absent in this container) and register
    the ctypes NTFF profiling hook against libaxon_pjrt.so, so
    run_bass_kernel_spmd(trace=True) can capture hardware profiles."""
    import sys
    import types

    if "antenv.axon_hooks" in sys.modules:
        return
    try:
        import antenv
        from trn_agent_boot.trn_boot import _ntff_profile_via_ctypes

        hook = _ntff_profile_via_ctypes("/opt/axon/libaxon_pjrt.so")
        mod = types.ModuleType("antenv.axon_hooks")
        _state = {"hook": hook}
        mod.set_axon_ntff_profile_hook = lambda h: _state.__setitem__("hook", h)
        mod.get_axon_ntff_profile_hook = lambda: _state["hook"]
        sys.modules["antenv.axon_hooks"] = mod
        antenv.axon_hooks = mod
    except Exception as e:  # profiling is best-effort
        print(f"ntff hook install failed: {e}", file=sys.stderr)


def _program(tc, x_in, wkq, wv, wp, bkq, mb, y):
    nc = tc.nc
    Exp = mybir.ActivationFunctionType.Exp

    const = tc.alloc_tile_pool(name="const", bufs=1)
    big = tc.alloc_tile_pool(name="big", bufs=1)
    expp = tc.alloc_tile_pool(name="expp", bufs=8)
    dnm = tc.alloc_tile_pool(name="dnm", bufs=4)
    rcpp = tc.alloc_tile_pool(name="rcpp", bufs=2)
    ostg = tc.alloc_tile_pool(name="ostg", bufs=4)
    psum = tc.alloc_tile_pool(name="psum", bufs=4, space="PSUM")

    # ---- load weights / constants
    # x arrives host-transposed ([D, S]); split the load across both HWDGE
    # queues so it lands fast.
    # tiny tensors first so they don't queue behind the bulk transfers
    bkq_sb = const.tile([P, 2 * NPAIR], F32)
    nc.sync.dma_start(bkq_sb, bkq)
    mb_sb = const.tile([P, NKB], F32)
    nc.sync.dma_start(mb_sb, mb)
    wkq_sb = const.tile([P, DC, 2 * DL], F16)
    nc.scalar.dma_start(wkq_sb, wkq.rearrange("(dc p) j -> p dc j", p=P))
    # x loaded by s-chunk so the first K/Q projection groups (which need all
    # d-chunks but only one s-range) unblock as early as possible
    xT_sb = const.tile([P, DC, S], F16)
    xr = x_in.rearrange("(dc p) s -> p dc s", p=P)
    for sc in range(4):
        for h2 in range(2):
            eng = nc.sync if h2 == 0 else nc.scalar
            eng.dma_start(
                xT_sb[:, h2 * 4:(h2 + 1) * 4, sc * 512:(sc + 1) * 512],
                xr[:, h2 * 4:(h2 + 1) * 4, sc * 512:(sc + 1) * 512])
    wv_sb = const.tile([P, DC, DL], F16)
    nc.sync.dma_start(wv_sb, wv.rearrange("(dc p) j -> p dc j", p=P))
    wp_sb = const.tile([P, DCL, D], F16)
    nc.sync.dma_start(wp_sb, wp.rearrange("(dc p) j -> p dc j", p=P))
    ones_sb = const.tile([P, E], F16)
    nc.vector.memset(ones_sb, 1.0)
    warm_sb = const.tile([P, 512], F16)
    nc.vector.memset(warm_sb, 0.5)

    kT_sb = big.tile([P, NPAIR, S], F16)
    qT_sb = big.tile([P, NPAIR, S], F16)
    v_sb = big.tile([P, NKB, DL], F16)
    yT_sb = big.tile([P, DCL, S], F16)

    def kq_halves(pi, which, st):
        tgt = kT_sb if which == 0 else qT_sb
        jb = which * NPAIR + pi
        cell = {}

        def mms(dcs, first, last):
            if first:
                cell["ps"] = psum.tile([P, QT], F32, tag="ps", name="ps")
            ps = cell["ps"]
            for dc in dcs:
                nc.tensor.matmul(
                    ps[:, :512],
                    lhsT=wkq_sb[:, dc, jb * P:(jb + 1) * P],
                    rhs=xT_sb[:, dc, st * 512:(st + 1) * 512],
                    start=(dc == dcs[0] and first), stop=(dc == dcs[-1] and last),
                )
            if last:
                nc.vector.tensor_scalar_add(
                    tgt[:, pi, st * 512:(st + 1) * 512], ps[:, :512],
                    bkq_sb[:, jb:jb + 1],
                )
        return [lambda: mms(list(range(4)), True, False),
                lambda: mms(list(range(4, 8)), False, True)]

    def v_halves(sb):
        cell = {}

        def mms(dcs, first, last):
            if first:
                cell["ps"] = psum.tile([P, QT], F32, tag="ps", name="ps")
            ps = cell["ps"]
            for dc in dcs:
                nc.tensor.matmul(
                    ps[:, :DL],
                    lhsT=xT_sb[:, dc, sb * P:(sb + 1) * P],
                    rhs=wv_sb[:, dc, :],
                    start=(dc == dcs[0] and first), stop=(dc == dcs[-1] and last),
                )
            if last:
                nc.vector.tensor_copy(v_sb[:, sb, :], ps[:, :DL])
        return [lambda: mms(list(range(4)), True, False),
                lambda: mms(list(range(4, 8)), False, True)]

    def proj_group(sb, ni):
        def go():
            ps = psum.tile([P, QT], F32, tag="ps", name="ps")
            for dc in range(DCL):
                nc.tensor.matmul(
                    ps[:, :512],
                    lhsT=yT_sb[:, dc, sb * P:(sb + 1) * P],
                    rhs=wp_sb[:, dc, ni * 512:(ni + 1) * 512],
                    start=(dc == 0), stop=(dc == DCL - 1),
                )
            stg = ostg.tile([P, 512], F32, tag="stg", name="st")
            nc.vector.tensor_copy(stg, ps[:, :512])
            nc.sync.dma_start(y[sb * P:(sb + 1) * P, ni * 512:(ni + 1) * 512], stg)
        return go

    def kq_groups(pi):
        # ordered so attention (qt 0) unblocks after the first 3 groups
        order = [(0, 0), (1, 0), (1, 1), (0, 1), (0, 2), (0, 3), (1, 2), (1, 3)]
        out = []
        for w, st in order:
            out += kq_halves(pi, w, st)
        return out

    # Warm up the PE clock (HAM) while the input DMAs land, then emit only
    # the minimal K/Q groups needed for the first q-tile; everything else
    # (rest of pair-0 K/Q, V projection, later pairs' K/Q, first half of the
    # output projection) is injected into the attention kb-loops so its
    # PSUM-slot requests interleave with the attention tiles' FIFO instead
    # of serializing at phase boundaries.
    wps = psum.tile([P, QT], F32, tag="ps", name="ps")
    for _ in range(16):
        nc.tensor.matmul(wps[:E, :512], lhsT=ones_sb[:, :E],
                         rhs=warm_sb[:, :512], start=True, stop=True)

    kq0 = {(w, st): kq_halves(0, w, st) for w in (0, 1) for st in range(4)}
    for w, st in ((0, 0), (1, 0), (1, 1)):
        for g in kq0[(w, st)]:
            g()

    for pi in range(NPAIR):
        for qi in range(S // QT):
            slots = [[] for _ in range(NKB)]

            def place(items, kb):
                slots[kb].extend(items)

            def spread(items):
                n = len(items)
                for j, it in enumerate(items):
                    slots[j * NKB // n].append(it)

            if pi == 0:
                if qi == 0:
                    for sb in range(NKB):
                        place(v_halves(sb), sb)
                    # just-in-time remainder of pair-0 K/Q (k-st j gates kb 4j)
                    ka, kb_ = kq0[(0, 1)]; place([ka], 2); place([kb_], 3)
                    ka, kb_ = kq0[(0, 2)]; place([ka], 6); place([kb_], 7)
                    ka, kb_ = kq0[(0, 3)]; place([ka], 10); place([kb_], 11)
                    place(kq0[(1, 2)], 12)
                    place(kq0[(1, 3)], 14)
                else:
                    spread(kq_groups(1))
            elif pi < NPAIR - 1:
                halves = kq_groups(pi + 1)
                spread(halves[:8] if qi == 0 else halves[8:])
            if pi == NPAIR - 1 and qi == 1:
                spread([proj_group(sb, ni) for sb in range(8) for ni in range(2)])
            q0 = qi * QT
            pv_ps = psum.tile([P, QT], F32, tag="ps", name="ps")
            accs = [dnm.tile([P, QT], F16, tag="dnm", name="dn") for _ in range(2)]
            for kb in range(NKB):
                for it in slots[kb]:
                    it()
                stps = []
                for h in range(2):
                    lo = h * E
                    stp = psum.tile([P, QT], F32, tag="ps", name="ps")
                    for n in range(QT // 512):
                        nc.tensor.matmul(
                            stp[:, n * 512:(n + 1) * 512],
                            lhsT=kT_sb[lo:lo + E, pi, kb * P:(kb + 1) * P],
                            rhs=qT_sb[lo:lo + E, pi, q0 + n * 512:q0 + (n + 1) * 512],
                            start=True, stop=True,
                        )
                    stps.append(stp)
                for h in range(2):
                    lo = h * E
                    ex = expp.tile([P, QT], F16, tag="exp", name="ex")
                    nc.scalar.activation(ex, stps[h], Exp,
                                         bias=mb_sb[:, kb:kb + 1], scale=0.125)
                    for n in range(QT // 512):
                        nc.tensor.matmul(
                            pv_ps[lo:lo + E, n * 512:(n + 1) * 512],
                            lhsT=v_sb[:, kb, pi * P + lo: pi * P + lo + E],
                            rhs=ex[:, n * 512:(n + 1) * 512],
                            start=(kb == 0), stop=(kb == NKB - 1),
                        )
                    # softmax denominator: one smooth in-place add per chunk
                    if kb == 0:
                        nc.vector.tensor_copy(accs[h], ex)
                    else:
                        nc.vector.tensor_add(accs[h], accs[h], ex)
            bd_ps = psum.tile([P, QT], F32, tag="ps", name="ps")
            for h in range(2):
                lo = h * E
                for n in range(QT // 512):
                    nc.tensor.matmul(
                        bd_ps[lo:lo + E, n * 512:(n + 1) * 512],
                        lhsT=ones_sb[:, :E],
                        rhs=accs[h][:, n * 512:(n + 1) * 512],
                        start=True, stop=True,
                    )
            rcp = rcpp.tile([P, QT], F32, tag="rcp", name="rc")
            nc.vector.reciprocal_approx_fast(rcp, bd_ps)
            nc.vector.tensor_mul(yT_sb[:, pi, q0:q0 + QT], pv_ps, rcp)

    # ---- remaining output projection (sb 0..7 was injected above)
    for sb in range(8, NKB):
        for ni in range(D // 512):
            proj_group(sb, ni)()

    for pool in (psum, ostg, rcpp, dnm, expp, big, const):
        pool.release()


def _build():
    if "nc" in _CACHE:
        return _CACHE["nc"]
    nc = bacc.Bacc("TRN2", target_bir_lowering=False, debug=False)
    x_in = nc.dram_tensor("x_in", (D, S), F16, kind="ExternalInput")
    wkq = nc.dram_tensor("wkq", (D, 2 * DL), F16, kind="ExternalInput")
    wv = nc.dram_tensor("wv", (D, DL), F16, kind="ExternalInput")
    wp = nc.dram_tensor("wp", (DL, D), F16, kind="ExternalInput")
    bkq = nc.dram_tensor("bkq", (P, 2 * NPAIR), F32, kind="ExternalInput")
    mb = nc.dram_tensor("mb", (P, NKB), F32, kind="ExternalInput")
    y = nc.dram_tensor("y", (S, D), F32, kind="ExternalOutput")
    with tile.TileContext(nc) as tc:
        _program(tc, x_in.ap(), wkq.ap(), wv.ap(), wp.ap(), bkq.ap(), mb.ap(), y.ap())
    nc.compile()
    _CACHE["nc"] = nc
    return nc


def kernel(x, attention_mask, W_qkv, b_qkv, W_proj, b_proj):
    global LAST_RESULTS
    x = np.asarray(x, dtype=np.float32)
    attention_mask = np.asarray(attention_mask, dtype=bool)
    W_qkv = np.asarray(W_qkv, dtype=np.float32)
    b_qkv = np.asarray(b_qkv, dtype=np.float32)
    W_proj = np.asarray(W_proj, dtype=np.float32)
    b_proj = np.asarray(b_proj, dtype=np.float32)

    nc = _build()

    xT16 = np.ascontiguousarray(
        x.astype(np.float16).transpose(0, 2, 1))                  # [B, D, S]
    maskb = np.where(attention_mask, 0.0, -1e9).astype(np.float32)  # [B, S]

    wkq_g, wv_g, wp_g, bkq_g = [], [], [], []
    for g in range(G):
        wk = W_qkv[DL * g:DL * (g + 1)]                    # [DL, D]
        wq = W_qkv[D + DL * g:D + DL * (g + 1)]
        wvl = W_qkv[2 * D + DL * g:2 * D + DL * (g + 1)]
        wkq_g.append(np.ascontiguousarray(
            np.concatenate([wk, wq], axis=0).T).astype(np.float16))   # [D, 2*DL]
        wv_g.append(np.ascontiguousarray(wvl.T).astype(np.float16))   # [D, DL]
        wp_g.append(np.ascontiguousarray(
            W_proj.T[DL * g:DL * (g + 1)]).astype(np.float16))        # [DL, D]
        bk = b_qkv[DL * g:DL * (g + 1)]
        bq = b_qkv[D + DL * g:D + DL * (g + 1)]
        bkq_g.append(np.ascontiguousarray(
            np.concatenate([bk, bq]).reshape(2 * NPAIR, P).T).astype(np.float32))

    in_maps = []
    for c in range(2 * B // 1 * 1)[:8]:
        b, g = c // G, c % G
        in_maps.append({
            "x_in": xT16[b],
            "wkq": wkq_g[g],
            "wv": wv_g[g],
            "wp": wp_g[g],
            "bkq": bkq_g[g],
            "mb": np.ascontiguousarray(maskb[b].reshape(NKB, P).T),
        })

    trace = os.environ.get("KERNEL_TRACE", "0") == "1"
    if trace:
        _install_ntff_hook()
    LAST_RESULTS = run_bass_kernel_spmd(
        nc, in_maps, core_ids=list(range(8)), trace=trace,
        trace_cores=list(range(8)), stitch_traces=False,
    )
    results = LAST_RESULTS.results

    bv = b_qkv[2 * D:]
    cvec = (bv @ W_proj.T + b_proj).astype(np.float32)            # [D]
    out = np.empty((B, S, D), np.float32)
    for b in range(B):
        out[b] = results[G * b]["y"] + results[G * b + 1]["y"] + cvec
    return out
